# revision 120
# baseline (speedup 1.0000x reference)
"""LocalScoreMachine Trainium2 kernel (pixel-major frontend + PE box filter).

score[b,c,p] = (m*SWI_c/SW - x[b,c,p]) / sig2, where over the dataset axis n:
    SW  = sum_n w,  SWI_c = sum_n w*I_c,
    w   = exp(box3(u)),  u = sum_c I_c * x'_c + c_s*S,
    x'  = x*(m/sig2),  S = sum_c I_c^2,  c_s = -m^2/(2*sig2).
(The b- and n-constant exp factors cancel in the SWI/SW ratio, and the
3x3 box filter is linear, so box3(u) gives the exact exponent up to that
constant.)

Sharding: dataset axis N=2048 -> 256 images per core (8 cores).

Per-core dataflow:
  *P-major phase* (partitions = 128 pixels of a 4-row chunk, free = n):
    - t_c = I_c * x'_c via tensor_scalar muls (per-partition scalar = x'),
      which hit the DVE 4x perf mode in bf16.  Split across DVE/Act/Pool.
    - u = t0+t1+t2+S'' via wide tensor adds (free = all 8 chunks * 256 n).
    - box3 via PE matmuls: block-tridiagonal 0/1 band matrices B_delta
      [128x128] couple chunk ci with chunks ci+delta; zero guard slots
      implement the zero padding.  Accumulated in PSUM.
    - w = exp(arg) on the Act engine, PSUM -> SBUF bf16.
  *Transpose*: one SBUF->SBUF dma_start_transpose per b flips w to
    image-major layout [n partitions, pixel free].
  *A-major phase* (partitions = n): v_c = w*I_c muls, then ones-matmul
    reductions over n on the PE into PSUM, accumulated over both n-tiles.
Host combines the 8 partial (SW, SWI) results and forms the score.
"""

import sys

for _p in ("/opt/trn_rl_repo", "/opt/trn_rl_repo/concourse", "/opt/pypackages"):
    if _p not in sys.path:
        sys.path.append(_p)

from contextlib import ExitStack

import numpy as np
import ml_dtypes

import concourse.bass as bass
import concourse.bacc as bacc
import concourse.mybir as mybir
import concourse.tile as tile
from concourse import bass_utils

B, N, C, H, W = 8, 2048, 3, 32, 32
P = H * W  # 1024 pixels
NCORES = 8
NLOC = N // NCORES  # 256 images per core
NT = 2  # n-tiles (128 partitions) per core on the A-major side
CI = 8  # pixel chunks (4 h-rows x 32 w = 128 pixels each)
Q = 128  # pixels per chunk
F32 = mybir.dt.float32
BF16 = mybir.dt.bfloat16
AF = mybir.ActivationFunctionType
BF_NP = ml_dtypes.bfloat16

_cache = {}
_last_res = None

# Engine assignment for the 24 per-b tensor_scalar muls (c, ci), tuned so
# DVE/Act/Pool finish together (DVE ~127ns/op, Act ~398, Pool ~451).
# index = c * CI + ci -> "d" (DVE) / "a" (Act) / "p" (Pool)
# Early iterations (b<2) weight the muls toward DVE/Act so the first u
# tiles are produced quickly (PE would otherwise starve at startup); later
# iterations push work to the otherwise-idle Pool engine.
_MUL_ENG_EARLY = list("ddadddpd" "adddddpa" "ddaddpdd")
_MUL_ENG = list("apapapdp" "apdpaapd" "pdapaapd")


def _box_mats():
    """B_delta[k, m] = 1 if source pixel k (in chunk ci+delta) is in the
    3x3 neighborhood of target pixel m (in chunk ci); mats[3] = identity
    (used to seed the PSUM accumulation with the precomputed Bs term)."""
    q = np.arange(Q)
    hk, wk = q // W, q % W  # chunk-local h (0..3), w (0..31)
    hm, wm = hk, wk
    mats = []
    for delta in (-1, 0, 1):
        dh = 4 * delta + hk[:, None] - hm[None, :]
        dw = wk[:, None] - wm[None, :]
        mats.append(((np.abs(dh) <= 1) & (np.abs(dw) <= 1)).astype(np.float32))
    mats.append(np.eye(Q, dtype=np.float32))
    return np.stack(mats)  # [4, 128, 128] (k, m)


def _box3(a):
    """3x3 zero-padded box sum over the last two dims."""
    Hh, Ww = a.shape[-2], a.shape[-1]
    p = np.pad(a, [(0, 0)] * (a.ndim - 2) + [(1, 1), (1, 1)])
    return sum(
        p[..., i : i + Hh, j : j + Ww] for i in range(3) for j in range(3)
    )


def _build():
    nc = bacc.Bacc("TRN2", target_bir_lowering=False, debug=False)

    ip_d = nc.dram_tensor("ip", [Q, C, CI, NLOC], BF16, kind="ExternalInput")
    ia_d = nc.dram_tensor("ia", [NT, 128, C, CI, Q], BF16, kind="ExternalInput")
    bs_d = nc.dram_tensor("bs", [Q, CI, NLOC], BF16, kind="ExternalInput")
    xsc_d = nc.dram_tensor("xsc", [Q, B, C, CI], F32, kind="ExternalInput")
    bm_d = nc.dram_tensor("bm", [4, Q, Q], BF16, kind="ExternalInput")
    out_d = nc.dram_tensor("out", [B, 9, 512], F32, kind="ExternalOutput")

    with tile.TileContext(nc) as tc, ExitStack() as ctx:
        const = ctx.enter_context(tc.tile_pool(name="const", bufs=1))
        work = ctx.enter_context(tc.tile_pool(name="work", bufs=3))
        wpool = ctx.enter_context(tc.tile_pool(name="wpool", bufs=5))
        psum = ctx.enter_context(
            tc.tile_pool(name="psum", bufs=2, space=bass.MemorySpace.PSUM)
        )
        rpsum = ctx.enter_context(
            tc.tile_pool(name="rpsum", bufs=2, space=bass.MemorySpace.PSUM)
        )

        # Load order matters: the b=0 muls need xst + ipt first.
        xst = const.tile([Q, B, C, CI], F32)
        nc.gpsimd.dma_start(xst[:], xsc_d.ap())
        ipt = const.tile([Q, C, CI, NLOC], BF16)
        for c in range(C):
            nc.sync.dma_start(ipt[:, c, 0:4], ip_d.ap()[:, c, 0:4])
        bmt = []
        for d in range(4):
            bm = const.tile([Q, Q], BF16, name=f"bm{d}")
            nc.sync.dma_start(bm[:], bm_d.ap()[d])
            bmt.append(bm)
        bst = const.tile([Q, CI, NLOC], BF16)
        nc.sync.dma_start(bst[:, 0:4], bs_d.ap()[:, 0:4])
        for c in range(C):
            nc.sync.dma_start(ipt[:, c, 4:8], ip_d.ap()[:, c, 4:8])
        nc.sync.dma_start(bst[:, 4:8], bs_d.ap()[:, 4:8])
        iat = []
        for t in range(NT):
            it = const.tile([128, C, CI, Q], BF16, name=f"ia{t}")
            # split so the first w-transposes can interleave on the DMA device
            nc.sync.dma_start(it[:, 0:2], ia_d.ap()[t][:, 0:2])
            nc.sync.dma_start(it[:, 2:3], ia_d.ap()[t][:, 2:3])
            iat.append(it)
        ones = const.tile([128, 32], BF16)
        nc.gpsimd.memset(ones[:], 1.0)

        # u tiles with zero guard slots 0 and CI+1 (zero padding for box3)
        ue = []
        for i in range(4):
            u = const.tile([Q, CI + 2, NLOC], BF16, name=f"ue{i}")
            nc.gpsimd.memset(u[:, 0], 0.0)
            nc.gpsimd.memset(u[:, CI + 1], 0.0)
            ue.append(u)

        was = {}

        def front(b):
            u = ue[b % 4]
            # t_c = I_c * x'_c  (P-major; per-partition scalar muls)
            tcs = work.tile([Q, C, CI, NLOC], BF16, tag="tcs")
            z01 = work.tile([Q, CI, NLOC], BF16, tag="z01")

            def mul_one(c, ci):
                eng = (_MUL_ENG_EARLY if b < 2 else _MUL_ENG)[c * CI + ci]
                dst = tcs[:, c, ci]
                src = ipt[:, c, ci]
                sc = xst[:, b, c, ci : ci + 1]
                if eng == "d":
                    nc.vector.tensor_scalar_mul(dst, src, sc)
                elif eng == "a":
                    nc.scalar.mul(dst, src, sc)
                else:
                    nc.gpsimd.tensor_scalar_mul(dst, src, sc)

            if b == 0:
                # half-granular: first u half is ready before the second
                # third of the image DMA lands
                for hh in range(2):
                    hs = slice(4 * hh, 4 * hh + 4)
                    for c in range(C):
                        for ci in range(4 * hh, 4 * hh + 4):
                            mul_one(c, ci)
                    nc.vector.tensor_add(
                        z01[:, hs], tcs[:, 0, hs], tcs[:, 1, hs]
                    )
                    nc.vector.tensor_add(
                        u[:, 1 + 4 * hh : 5 + 4 * hh], z01[:, hs],
                        tcs[:, 2, hs],
                    )
            else:
                for c in range(C):
                    for ci in range(CI):
                        mul_one(c, ci)
                nc.vector.tensor_add(z01[:], tcs[:, 0], tcs[:, 1])
                nc.vector.tensor_add(u[:, 1 : CI + 1], z01[:], tcs[:, 2])

            # box3 on PE: arg[ci] = Bs[ci] + sum_delta B_delta . u[ci+delta]
            wp = wpool.tile([Q, CI, NLOC], BF16, tag="wp")
            wa = wpool.tile([128, CI * NT, 128], BF16, tag="wa")
            for quarter in range(4):
                base = 1 + 2 * quarter  # u slot of first output chunk
                ap_ = psum.tile([Q, 2, NLOC], F32, tag="argp")
                dst = ap_[:].rearrange("q a n -> q (a n)")
                nc.tensor.matmul(
                    dst,
                    bmt[3][:],
                    bst[:, 2 * quarter : 2 * quarter + 2].rearrange(
                        "q a n -> q (a n)"
                    ),
                    start=True,
                    stop=False,
                )
                for di in range(3):
                    s = base + di - 1
                    nc.tensor.matmul(
                        dst,
                        bmt[di][:],
                        u[:, s : s + 2].rearrange("q a n -> q (a n)"),
                        start=False,
                        stop=(di == 2),
                    )
                nc.scalar.activation(
                    wp[:, 2 * quarter : 2 * quarter + 2].rearrange(
                        "q a n -> q (a n)"
                    ),
                    dst,
                    AF.Exp,
                )
                if quarter % 2 == 1:
                    # transpose this pixel-half of w to A-major:
                    # wa[nl, ci*2+t, q] = wp[q, ci, t*128+nl]
                    h = quarter // 2
                    nc.sync.dma_start_transpose(
                        wa[:, 8 * h : 8 * h + 8],
                        wp[:, 4 * h : 4 * h + 4].rearrange(
                            "q ci n -> q (ci n)"
                        ),
                    )
            was[b] = wa

        def back(b):
            wav = was.pop(b)[:].rearrange("nl (ci t) q -> nl t ci q", t=NT)

            # v_c = w * I_c and ones-matmul reductions over n.
            # Quantity r = 2*qnt+half lands in PSUM region
            # rows [32*(r//3), +32), free [512*(r%3), +512) (redundant rows).
            rp = rpsum.tile([96, 3, 512], F32, tag="red")

            def red_out(r):
                pr = 32 * (r // 3)
                return rp[pr : pr + 32, r % 3].rearrange(
                    "p (a q) -> p a q", a=4
                )

            for t in range(NT):
                first, last = (t == 0), (t == NT - 1)
                for half in range(2):
                    cs_ = slice(4 * half, 4 * half + 4)
                    wv = wav[:, t, cs_]  # [128, 4, Q] strided
                    wvb = wv.rearrange(
                        "nl (a one) q -> nl one a q", one=1
                    ).broadcast_to([128, C, 4, Q])
                    v = work.tile([128, C, 4, Q], BF16, tag="v")
                    nc.vector.tensor_mul(v[:], wvb, iat[t][:, :, cs_])
                    nc.tensor.matmul(
                        red_out(half), ones[:], wv,
                        start=first, stop=last,
                    )
                    for c in range(C):
                        nc.tensor.matmul(
                            red_out(2 + 2 * c + half), ones[:], v[:, c],
                            start=first, stop=last,
                        )
            # rows (0,32,64) x free-thirds -> out[b, 0:9, :] (row 8 unused)
            osb = work.tile([96, 3, 512], F32, tag="osb")
            if b == B - 1:
                nc.scalar.copy(
                    osb[:, 0:2].rearrange("g f q -> g (f q)"),
                    rp[:, 0:2].rearrange("g f q -> g (f q)"),
                )
                nc.vector.tensor_copy(osb[:, 2], rp[:, 2])
            else:
                nc.scalar.copy(
                    osb[:].rearrange("g f q -> g (f q)"),
                    rp[:].rearrange("g f q -> g (f q)"),
                )
            nc.sync.dma_start(
                out_d.ap()[b].rearrange("(g f) q -> g f q", f=3),
                osb[:].rearrange("(g p) f q -> g p f q", p=32)[:, 0],
            )

        # Software pipeline: the A-major back half of iteration b runs one
        # step behind its front half so the w-transpose DMA round trip is
        # hidden behind the next iteration's front-half work.
        for step in range(B + 2):
            if step < B:
                front(step)
            if step >= 2:
                back(step - 2)

    nc.compile()
    return nc


def kernel(x, images, mu, sigma, t):
    x = np.ascontiguousarray(np.asarray(x, dtype=np.float32))
    images = np.ascontiguousarray(np.asarray(images, dtype=np.float32))
    m = float(np.asarray(mu)[int(t)])
    sig = float(np.asarray(sigma)[int(t)])
    sig2 = sig * sig
    c_s = -(m * m) / (2.0 * sig2)

    key = ()
    if key not in _cache:
        _cache[key] = _build()
    nc = _cache[key]

    xp = x.reshape(B, C, P) * (m / sig2)
    # xsc[q, b, c, ci] = x'[b, c, ci*128+q]
    xsc = np.ascontiguousarray(
        xp.reshape(B, C, CI, Q).transpose(3, 0, 1, 2), dtype=np.float32
    )
    bmats = _box_mats().astype(BF_NP)

    imgs = images.reshape(N, C, P)
    in_maps = []
    for k in range(NCORES):
        il = imgs[k * NLOC : (k + 1) * NLOC]  # [256, 3, 1024]
        ilb = il.astype(BF_NP)
        ip = np.ascontiguousarray(
            ilb.reshape(NLOC, C, CI, Q).transpose(3, 1, 2, 0)
        )  # [q, c, ci, n]
        ia = np.ascontiguousarray(ilb.reshape(NT, 128, C, CI, Q))
        s2 = c_s * (il.astype(np.float64) ** 2).sum(axis=1)  # [256, 1024]
        bsv = _box3(s2.reshape(NLOC, H, W)).reshape(NLOC, P)
        bs = np.ascontiguousarray(
            bsv.reshape(NLOC, CI, Q).transpose(2, 1, 0).astype(BF_NP)
        )  # [q, ci, n]
        in_maps.append({"ip": ip, "ia": ia, "bs": bs, "xsc": xsc, "bm": bmats})

    import os

    trace = bool(os.environ.get("KERNEL_TRACE"))
    res = bass_utils.run_bass_kernel_spmd(
        nc, in_maps, core_ids=list(range(NCORES)), trace=trace
    )
    global _last_res
    _last_res = res
    parts = np.stack(
        [res.results[k]["out"] for k in range(NCORES)]
    )  # [8, B, 8, 512]
    tot = parts.astype(np.float64).sum(axis=0)  # [B, 8, 512]
    sw = tot[:, 0:2].reshape(B, P)
    swi = tot[:, 2:8].reshape(B, C, P)
    score = (m * swi / sw[:, None, :] - x.reshape(B, C, P)) / sig2
    return score.reshape(B, C, H, W).astype(np.float32)


# revision 121
# speedup vs baseline: 1.0231x; 1.0231x over previous
"""LocalScoreMachine Trainium2 kernel (pixel-major frontend + PE box filter).

score[b,c,p] = (m*SWI_c/SW - x[b,c,p]) / sig2, where over the dataset axis n:
    SW  = sum_n w,  SWI_c = sum_n w*I_c,
    w   = exp(box3(u)),  u = sum_c I_c * x'_c + c_s*S,
    x'  = x*(m/sig2),  S = sum_c I_c^2,  c_s = -m^2/(2*sig2).
(The b- and n-constant exp factors cancel in the SWI/SW ratio, and the
3x3 box filter is linear, so box3(u) gives the exact exponent up to that
constant.)

Sharding: dataset axis N=2048 -> 256 images per core (8 cores).

Per-core dataflow:
  *P-major phase* (partitions = 128 pixels of a 4-row chunk, free = n):
    - t_c = I_c * x'_c via tensor_scalar muls (per-partition scalar = x'),
      which hit the DVE 4x perf mode in bf16.  Split across DVE/Act/Pool.
    - u = t0+t1+t2+S'' via wide tensor adds (free = all 8 chunks * 256 n).
    - box3 via PE matmuls: block-tridiagonal 0/1 band matrices B_delta
      [128x128] couple chunk ci with chunks ci+delta; zero guard slots
      implement the zero padding.  Accumulated in PSUM.
    - w = exp(arg) on the Act engine, PSUM -> SBUF bf16.
  *Transpose*: one SBUF->SBUF dma_start_transpose per b flips w to
    image-major layout [n partitions, pixel free].
  *A-major phase* (partitions = n): v_c = w*I_c muls, then ones-matmul
    reductions over n on the PE into PSUM, accumulated over both n-tiles.
Host combines the 8 partial (SW, SWI) results and forms the score.
"""

import sys

for _p in ("/opt/trn_rl_repo", "/opt/trn_rl_repo/concourse", "/opt/pypackages"):
    if _p not in sys.path:
        sys.path.append(_p)

from contextlib import ExitStack

import numpy as np
import ml_dtypes

import concourse.bass as bass
import concourse.bacc as bacc
import concourse.mybir as mybir
import concourse.tile as tile
from concourse import bass_utils

B, N, C, H, W = 8, 2048, 3, 32, 32
P = H * W  # 1024 pixels
NCORES = 8
NLOC = N // NCORES  # 256 images per core
NT = 2  # n-tiles (128 partitions) per core on the A-major side
CI = 8  # pixel chunks (4 h-rows x 32 w = 128 pixels each)
Q = 128  # pixels per chunk
F32 = mybir.dt.float32
BF16 = mybir.dt.bfloat16
AF = mybir.ActivationFunctionType
BF_NP = ml_dtypes.bfloat16

_cache = {}
_last_res = None

# Engine assignment for the 24 per-b tensor_scalar muls (c, ci), tuned so
# DVE/Act/Pool finish together (DVE ~127ns/op, Act ~398, Pool ~451).
# index = c * CI + ci -> "d" (DVE) / "a" (Act) / "p" (Pool)
# Early iterations (b<2) weight the muls toward DVE/Act so the first u
# tiles are produced quickly (PE would otherwise starve at startup); later
# iterations push work to the otherwise-idle Pool engine.
_MUL_ENG_EARLY = list("ddadddpd" "adddddpa" "ddaddpdd")
_MUL_ENG = list("dpapapdp" "apdpaapd" "pdapaapd")


def _box_mats():
    """B_delta[k, m] = 1 if source pixel k (in chunk ci+delta) is in the
    3x3 neighborhood of target pixel m (in chunk ci); mats[3] = identity
    (used to seed the PSUM accumulation with the precomputed Bs term)."""
    q = np.arange(Q)
    hk, wk = q // W, q % W  # chunk-local h (0..3), w (0..31)
    hm, wm = hk, wk
    mats = []
    for delta in (-1, 0, 1):
        dh = 4 * delta + hk[:, None] - hm[None, :]
        dw = wk[:, None] - wm[None, :]
        mats.append(((np.abs(dh) <= 1) & (np.abs(dw) <= 1)).astype(np.float32))
    mats.append(np.eye(Q, dtype=np.float32))
    return np.stack(mats)  # [4, 128, 128] (k, m)


def _box3(a):
    """3x3 zero-padded box sum over the last two dims."""
    Hh, Ww = a.shape[-2], a.shape[-1]
    p = np.pad(a, [(0, 0)] * (a.ndim - 2) + [(1, 1), (1, 1)])
    return sum(
        p[..., i : i + Hh, j : j + Ww] for i in range(3) for j in range(3)
    )


def _build():
    nc = bacc.Bacc("TRN2", target_bir_lowering=False, debug=False)

    ip_d = nc.dram_tensor("ip", [Q, C, CI, NLOC], BF16, kind="ExternalInput")
    ia_d = nc.dram_tensor("ia", [NT, 128, C, CI, Q], BF16, kind="ExternalInput")
    bs_d = nc.dram_tensor("bs", [Q, CI, NLOC], BF16, kind="ExternalInput")
    xsc_d = nc.dram_tensor("xsc", [Q, B, C, CI], F32, kind="ExternalInput")
    bm_d = nc.dram_tensor("bm", [4, Q, Q], BF16, kind="ExternalInput")
    out_d = nc.dram_tensor("out", [B, 9, 512], F32, kind="ExternalOutput")

    with tile.TileContext(nc) as tc, ExitStack() as ctx:
        const = ctx.enter_context(tc.tile_pool(name="const", bufs=1))
        work = ctx.enter_context(tc.tile_pool(name="work", bufs=3))
        wpool = ctx.enter_context(tc.tile_pool(name="wpool", bufs=5))
        psum = ctx.enter_context(
            tc.tile_pool(name="psum", bufs=2, space=bass.MemorySpace.PSUM)
        )
        rpsum = ctx.enter_context(
            tc.tile_pool(name="rpsum", bufs=2, space=bass.MemorySpace.PSUM)
        )

        # Load order matters: the b=0 muls need xst + ipt first.
        xst = const.tile([Q, B, C, CI], F32)
        nc.gpsimd.dma_start(xst[:], xsc_d.ap())
        ipt = const.tile([Q, C, CI, NLOC], BF16)
        for c in range(C):
            nc.sync.dma_start(ipt[:, c, 0:4], ip_d.ap()[:, c, 0:4])
        bmt = []
        for d in range(4):
            bm = const.tile([Q, Q], BF16, name=f"bm{d}")
            nc.sync.dma_start(bm[:], bm_d.ap()[d])
            bmt.append(bm)
        bst = const.tile([Q, CI, NLOC], BF16)
        nc.sync.dma_start(bst[:, 0:4], bs_d.ap()[:, 0:4])
        for c in range(C):
            nc.sync.dma_start(ipt[:, c, 4:8], ip_d.ap()[:, c, 4:8])
        nc.sync.dma_start(bst[:, 4:8], bs_d.ap()[:, 4:8])
        iat = []
        for t in range(NT):
            it = const.tile([128, C, CI, Q], BF16, name=f"ia{t}")
            # split so the first w-transposes can interleave on the DMA device
            nc.sync.dma_start(it[:, 0:2], ia_d.ap()[t][:, 0:2])
            nc.sync.dma_start(it[:, 2:3], ia_d.ap()[t][:, 2:3])
            iat.append(it)
        ones = const.tile([128, 32], BF16)
        nc.gpsimd.memset(ones[:], 1.0)

        # u tiles with zero guard slots 0 and CI+1 (zero padding for box3)
        ue = []
        for i in range(4):
            u = const.tile([Q, CI + 2, NLOC], BF16, name=f"ue{i}")
            nc.gpsimd.memset(u[:, 0], 0.0)
            nc.gpsimd.memset(u[:, CI + 1], 0.0)
            ue.append(u)

        was = {}

        def front(b):
            u = ue[b % 4]
            # t_c = I_c * x'_c  (P-major; per-partition scalar muls)
            tcs = work.tile([Q, C, CI, NLOC], BF16, tag="tcs")
            z01 = work.tile([Q, CI, NLOC], BF16, tag="z01")

            def mul_one(c, ci):
                eng = (_MUL_ENG_EARLY if b < 2 else _MUL_ENG)[c * CI + ci]
                dst = tcs[:, c, ci]
                src = ipt[:, c, ci]
                sc = xst[:, b, c, ci : ci + 1]
                if eng == "d":
                    nc.vector.tensor_scalar_mul(dst, src, sc)
                elif eng == "a":
                    nc.scalar.mul(dst, src, sc)
                else:
                    nc.gpsimd.tensor_scalar_mul(dst, src, sc)

            if b == 0:
                # half-granular: first u half is ready before the second
                # third of the image DMA lands
                for hh in range(2):
                    hs = slice(4 * hh, 4 * hh + 4)
                    for c in range(C):
                        for ci in range(4 * hh, 4 * hh + 4):
                            mul_one(c, ci)
                    nc.vector.tensor_add(
                        z01[:, hs], tcs[:, 0, hs], tcs[:, 1, hs]
                    )
                    nc.vector.tensor_add(
                        u[:, 1 + 4 * hh : 5 + 4 * hh], z01[:, hs],
                        tcs[:, 2, hs],
                    )
            else:
                for c in range(C):
                    for ci in range(CI):
                        mul_one(c, ci)
                nc.vector.tensor_add(z01[:], tcs[:, 0], tcs[:, 1])
                nc.vector.tensor_add(u[:, 1 : CI + 1], z01[:], tcs[:, 2])

            # box3 on PE: arg[ci] = Bs[ci] + sum_delta B_delta . u[ci+delta]
            wp = wpool.tile([Q, CI, NLOC], BF16, tag="wp")
            wa = wpool.tile([128, CI * NT, 128], BF16, tag="wa")
            for quarter in range(4):
                base = 1 + 2 * quarter  # u slot of first output chunk
                ap_ = psum.tile([Q, 2, NLOC], F32, tag="argp")
                dst = ap_[:].rearrange("q a n -> q (a n)")
                nc.tensor.matmul(
                    dst,
                    bmt[3][:],
                    bst[:, 2 * quarter : 2 * quarter + 2].rearrange(
                        "q a n -> q (a n)"
                    ),
                    start=True,
                    stop=False,
                )
                for di in range(3):
                    s = base + di - 1
                    nc.tensor.matmul(
                        dst,
                        bmt[di][:],
                        u[:, s : s + 2].rearrange("q a n -> q (a n)"),
                        start=False,
                        stop=(di == 2),
                    )
                nc.scalar.activation(
                    wp[:, 2 * quarter : 2 * quarter + 2].rearrange(
                        "q a n -> q (a n)"
                    ),
                    dst,
                    AF.Exp,
                )
                if quarter % 2 == 1:
                    # transpose this pixel-half of w to A-major:
                    # wa[nl, ci*2+t, q] = wp[q, ci, t*128+nl]
                    h = quarter // 2
                    nc.sync.dma_start_transpose(
                        wa[:, 8 * h : 8 * h + 8],
                        wp[:, 4 * h : 4 * h + 4].rearrange(
                            "q ci n -> q (ci n)"
                        ),
                    )
            was[b] = wa

        def back(b):
            wav = was.pop(b)[:].rearrange("nl (ci t) q -> nl t ci q", t=NT)

            # v_c = w * I_c and ones-matmul reductions over n.
            # Quantity r = 2*qnt+half lands in PSUM region
            # rows [32*(r//3), +32), free [512*(r%3), +512) (redundant rows).
            rp = rpsum.tile([96, 3, 512], F32, tag="red")

            def red_out(r):
                pr = 32 * (r // 3)
                return rp[pr : pr + 32, r % 3].rearrange(
                    "p (a q) -> p a q", a=4
                )

            for t in range(NT):
                first, last = (t == 0), (t == NT - 1)
                for half in range(2):
                    cs_ = slice(4 * half, 4 * half + 4)
                    wv = wav[:, t, cs_]  # [128, 4, Q] strided
                    wvb = wv.rearrange(
                        "nl (a one) q -> nl one a q", one=1
                    ).broadcast_to([128, C, 4, Q])
                    v = work.tile([128, C, 4, Q], BF16, tag="v")
                    nc.vector.tensor_mul(v[:], wvb, iat[t][:, :, cs_])
                    nc.tensor.matmul(
                        red_out(half), ones[:], wv,
                        start=first, stop=last,
                    )
                    for c in range(C):
                        nc.tensor.matmul(
                            red_out(2 + 2 * c + half), ones[:], v[:, c],
                            start=first, stop=last,
                        )
            # rows (0,32,64) x free-thirds -> out[b, 0:9, :] (row 8 unused)
            osb = work.tile([96, 3, 512], F32, tag="osb")
            if b == B - 1:
                nc.scalar.copy(
                    osb[:, 0:2].rearrange("g f q -> g (f q)"),
                    rp[:, 0:2].rearrange("g f q -> g (f q)"),
                )
                nc.vector.tensor_copy(osb[:, 2], rp[:, 2])
            else:
                nc.scalar.copy(
                    osb[:].rearrange("g f q -> g (f q)"),
                    rp[:].rearrange("g f q -> g (f q)"),
                )
            nc.sync.dma_start(
                out_d.ap()[b].rearrange("(g f) q -> g f q", f=3),
                osb[:].rearrange("(g p) f q -> g p f q", p=32)[:, 0],
            )

        # Software pipeline: the A-major back half of iteration b runs one
        # step behind its front half so the w-transpose DMA round trip is
        # hidden behind the next iteration's front-half work.
        for step in range(B + 2):
            if step < B:
                front(step)
            if step >= 2:
                back(step - 2)

    nc.compile()
    return nc


def kernel(x, images, mu, sigma, t):
    x = np.ascontiguousarray(np.asarray(x, dtype=np.float32))
    images = np.ascontiguousarray(np.asarray(images, dtype=np.float32))
    m = float(np.asarray(mu)[int(t)])
    sig = float(np.asarray(sigma)[int(t)])
    sig2 = sig * sig
    c_s = -(m * m) / (2.0 * sig2)

    key = ()
    if key not in _cache:
        _cache[key] = _build()
    nc = _cache[key]

    xp = x.reshape(B, C, P) * (m / sig2)
    # xsc[q, b, c, ci] = x'[b, c, ci*128+q]
    xsc = np.ascontiguousarray(
        xp.reshape(B, C, CI, Q).transpose(3, 0, 1, 2), dtype=np.float32
    )
    bmats = _box_mats().astype(BF_NP)

    imgs = images.reshape(N, C, P)
    in_maps = []
    for k in range(NCORES):
        il = imgs[k * NLOC : (k + 1) * NLOC]  # [256, 3, 1024]
        ilb = il.astype(BF_NP)
        ip = np.ascontiguousarray(
            ilb.reshape(NLOC, C, CI, Q).transpose(3, 1, 2, 0)
        )  # [q, c, ci, n]
        ia = np.ascontiguousarray(ilb.reshape(NT, 128, C, CI, Q))
        s2 = c_s * (il.astype(np.float64) ** 2).sum(axis=1)  # [256, 1024]
        bsv = _box3(s2.reshape(NLOC, H, W)).reshape(NLOC, P)
        bs = np.ascontiguousarray(
            bsv.reshape(NLOC, CI, Q).transpose(2, 1, 0).astype(BF_NP)
        )  # [q, ci, n]
        in_maps.append({"ip": ip, "ia": ia, "bs": bs, "xsc": xsc, "bm": bmats})

    import os

    trace = bool(os.environ.get("KERNEL_TRACE"))
    res = bass_utils.run_bass_kernel_spmd(
        nc, in_maps, core_ids=list(range(NCORES)), trace=trace
    )
    global _last_res
    _last_res = res
    parts = np.stack(
        [res.results[k]["out"] for k in range(NCORES)]
    )  # [8, B, 8, 512]
    tot = parts.astype(np.float64).sum(axis=0)  # [B, 8, 512]
    sw = tot[:, 0:2].reshape(B, P)
    swi = tot[:, 2:8].reshape(B, C, P)
    score = (m * swi / sw[:, None, :] - x.reshape(B, C, P)) / sig2
    return score.reshape(B, C, H, W).astype(np.float32)
